# revision 43
# baseline (speedup 1.0000x reference)
"""Patch-correlation argmax (retrieval KNN) on 8 Trainium2 NeuronCores.

Pipeline:
  host:   3x3 unfold of both images -> [B, 576, 9216] patch matrices,
          l2-normalize the ref patches, cast to fp8e4m3 in the DoubleRow
          interleave layout (K padded 576->768 = 3 chunks of 256).
  device: shard the lr-patch axis (n) across 8 cores (no collectives).
          Each core computes R tiles [128 n x 512 m] as 3 PSUM-accumulated
          fp8 DoubleRow matmuls, collapses each 1536-wide PSUM scan group
          to 32-element window maxes with one segmented tensor_reduce
          pass, and keeps the top-8 windows per m-row via MAX8 +
          FIND_INDEX8 over the 288 window maxes. R never touches HBM.
  host:   rescore all 8x32 member positions of the winning windows
          (fp32 batched gemm, then fp64 on the top 8) -> exact S (max
          corr) + Hmap (argmax index). Only the true argmax's WINDOW must
          reach the device top-8, so fp8 matmul noise never flips the
          final answer.
"""

import numpy as np

B, C, H, W = 2, 64, 96, 96
HW = H * W            # 9216 patches per image
K = C * 9             # 576 contraction dim
KC = 3                # K chunks of 256 (DoubleRow; 576 zero-padded to 768)
NCORES = 8
NPC = HW // NCORES    # 1152 lr patches per core
NT = NPC // 128       # 9 n-tiles of 128 per core
MTILE = 512           # matmul free dim / PSUM bank
MT = HW // MTILE      # 18 m-tiles
# m-scan groups: each 1536-element (3 PSUM banks) group is collapsed by a
# single segmented tensor_reduce(max) into 8-element window maxes. Uniform
# groups keep the PE-fill / DVE-scan cadence steady so PE never stalls past
# the ~3.4us HAM window (a throttled PE runs matmuls 2x slower).
GSIZES = (1536,) * 6
GBASES = tuple(1536 * i for i in range(6))
NG = len(GSIZES)
WIN = 32              # window size of the segmented max
NWIN_G = 1536 // WIN  # 48 window maxes per scan group
NWIN = HW // WIN      # 1152 window maxes per m-row
NTOP = 8              # top windows kept per pixel (MAX8 output width)
EPS = 1e-12


def _unfold(x):
    """[B,C,H,W] f32 -> [B, C*9, H*W] matching F.unfold(kernel=3, padding=1)."""
    xp = np.zeros((B, C, H + 2, W + 2), np.float32)
    xp[:, :, 1:-1, 1:-1] = x
    out = np.empty((B, C, 9, HW), np.float32)
    for di in range(3):
        for dj in range(3):
            out[:, :, di * 3 + dj] = xp[:, :, di:di + H, dj:dj + W].reshape(B, C, HW)
    return out.reshape(B, K, HW)


def _to_chunks_f8(u, scale):
    """[B, 576, HW'] f32 -> [B, 128, KC, 2, HW'] fp8e4m3, K padded to 768.

    DoubleRow layout: cell (p, kc, j) holds original k = kc*256 + j*128 + p.
    `scale` keeps small components out of the e4m3 subnormal range; it
    multiplies R uniformly, which the argmax pipeline never looks past.
    """
    import ml_dtypes
    n = u.shape[2]
    up = np.zeros((B, KC * 256, n), np.float32)
    up[:, :K] = u * scale
    return np.ascontiguousarray(
        up.reshape(B, KC, 2, 128, n).transpose(0, 3, 1, 2, 4)
    ).astype(ml_dtypes.float8_e4m3)


def _dedupe_ldweights(nc):
    """Drop an InstLdweights when the PE array already holds those weights.

    bass emits one LDWEIGHTS per matmul; with kc as the outer loop the same
    lhsT serves 4 consecutive matmuls, and LDW+MM serialize on the PE queue
    (~107ns each), so redundant loads cost real time. Weight state persists
    in the array across matmuls; any waits/updates on a dropped LDW migrate
    to the next PE instruction (later wait = same guarantee, later update =
    still-ordered signal).
    """
    import concourse.mybir as mybir

    for blk in nc.m.functions[0].blocks:
        out, last_sig, pending = [], None, []
        changed = False
        for ins in blk.instructions:
            nm = type(ins).__name__
            if nm == "InstLdweights":
                sig = str(ins.ins[0])
                if sig == last_sig:
                    if ins.sync_info is not None:
                        pending.append(ins.sync_info)
                    changed = True
                    continue
                last_sig = sig
            if pending and nm in ("InstMatmult", "InstLdweights"):
                si = ins.sync_info
                if si is None:
                    ins.sync_info = si = mybir.SyncInfo(on_wait=[], on_update=[])
                for p in pending:
                    si.on_wait.extend(p.on_wait)
                    si.on_update.extend(p.on_update)
                pending = []
            out.append(ins)
        assert not pending
        if changed:
            blk.instructions = out


def _cap_sync_waits(nc):
    """Walrus instruction structs hold a single sync-wait slot (matmul, DMA).

    Tile sometimes emits 2 waits on one instruction (input RAW + slot WAR).
    Split the excess into standalone EventSemaphore waits inserted directly
    before the instruction on the same engine queue — semantically identical
    (every wait still holds at the same queue position), always compilable.
    """
    import concourse.mybir as mybir

    n = 0
    for blk in nc.m.functions[0].blocks:
        ins_list = list(blk.instructions)
        out, changed = [], False
        for ins in ins_list:
            si = ins.sync_info
            if (si is not None and len(si.on_wait) > 1
                    and type(ins).__name__ != "InstEventSemaphore"):
                for w in si.on_wait[:-1]:
                    n += 1
                    out.append(mybir.InstEventSemaphore(
                        name=f"bridge_wait_{n}",
                        engine=ins.engine,
                        sync_info=mybir.SyncInfo(on_wait=[w], on_update=[]),
                    ))
                si.on_wait[:] = si.on_wait[-1:]
                changed = True
            out.append(ins)
        if changed:
            blk.instructions = out


def _build_bass(fix_waits=True):
    import concourse.bass as bass
    import concourse.mybir as mybir
    from concourse.tile import TileContext

    nc = bass.Bass()
    refk = nc.declare_dram_parameter(
        "refk", [B, 128, KC, 2, HW], mybir.dt.float8e4, isOutput=False)
    lrk = nc.declare_dram_parameter(
        "lrk", [B, 128, KC, 2, NPC], mybir.dt.float8e4, isOutput=False)
    vals = nc.declare_dram_parameter(
        "vals", [B, 128, NT, NTOP], mybir.dt.float32, isOutput=True)
    idxs = nc.declare_dram_parameter(
        "idxs", [B, 128, NT, NTOP], mybir.dt.uint16, isOutput=True)

    with TileContext(nc) as tc:
        with (
            tc.tile_pool(name="big", bufs=1) as big,
            tc.tile_pool(name="io", bufs=2) as io,
            tc.tile_pool(name="lrp", bufs=2) as lrp,
            tc.tile_pool(name="wmp", bufs=1) as wmp,
            tc.tile_pool(name="ps", bufs=2, space="PSUM") as psp,
            tc.tile_pool(name="psw", bufs=1, space="PSUM") as psw,
        ):
            # ~6.5us of dummy matmuls on memset data while the first input
            # DMAs stream in: PE activity releases the HAM clock throttle
            # (cold PE runs matmuls at 1.2GHz instead of 2.4GHz), so the
            # real matmuls start warm instead of paying ~20us of ramp
            scratch = big.tile([128, MTILE], mybir.dt.float16, tag="warm")
            nc.gpsimd.memset(scratch[:], 0.0)
            wps = psw.tile([128, MTILE], mybir.dt.float32, tag="wps")
            for _ in range(30):
                nc.tensor.matmul(wps, scratch[:, :128], scratch[:],
                                 start=True, stop=True)
            for b in range(B):
                # Tile's dependency tracking is per-tile, so ref gets one
                # tile per m-slice (single DMA writer each) rather than 18
                # partial writes into one big tile: matmuls on slice mt can
                # start as soon as that slice has landed.
                # per-kc lr tiles and interleaved issue order: the sync
                # DMA queue drains serially, so the first scan group's
                # operands (lr chunks + ref slices 0-2) must be queued
                # first or the PE sits idle ~13us at kernel start
                lr_ts = [lrp.tile([128, 2, NPC], mybir.dt.float8e4,
                                  tag=f"lr{kc}", name=f"lr{kc}")
                         for kc in range(KC)]
                ref_ts = [big.tile([128, KC, 2, MTILE], mybir.dt.float8e4,
                                   tag=f"ref{mt}", name=f"ref{mt}")
                          for mt in range(MT)]
                for i in range(MT):
                    if i < KC:
                        nc.sync.dma_start(lr_ts[i][:], lrk[b, :, i, :, :])
                    sl = slice(i * MTILE, (i + 1) * MTILE)
                    nc.sync.dma_start(ref_ts[i][:], refk[b, :, :, :, sl])
                # one batch-wide candidate buffer + a single out-DMA per
                # tensor: the out-DMA then carries exactly one (RAW) wait,
                # and batch 1 gets a fresh slot (bufs=2) so there is no WAR
                # wait on top — DMA structs also allow only one sync wait
                vt = io.tile([128, NT, NTOP], mybir.dt.float32, tag="vals")
                it = io.tile([128, NT, NTOP], mybir.dt.uint16, tag="idx")
                # window maxes live on SBUF; written by tensor_reduce and
                # read by MAX8/FIND_INDEX8, all on the DVE queue, so the
                # whole reduction chain needs no cross-engine semaphores
                wm = wmp.tile([128, NT, NWIN], mybir.dt.float32, tag="wm")
                for nt in range(NT):
                    for g in range(NG):
                        # 3-bank PSUM scan group: matmuls fill the group's
                        # 512-wide bank slices, then one wide MAX8 +
                        # FIND_INDEX8 pair scans all of it. kc is the outer
                        # loop so one LDWEIGHTS serves all 3 bank slices.
                        ps = psp.tile([128, 1536], mybir.dt.float32, tag="ps")
                        nslices = GSIZES[g] // MTILE
                        for kc in range(KC):
                            for s in range(nslices):
                                mt = GBASES[g] // MTILE + s
                                nc.tensor.matmul(
                                    ps[:, s * MTILE:(s + 1) * MTILE],
                                    lr_ts[kc][:, :, nt * 128:(nt + 1) * 128],
                                    ref_ts[mt][:, kc, :, :],
                                    start=(kc == 0),
                                    stop=(kc == KC - 1),
                                    perf_mode=mybir.MatmulPerfMode.DoubleRow,
                                )
                        # one pass over the PSUM group -> 8-wide window
                        # maxes (segmented reduce over the innermost axis)
                        nc.vector.tensor_reduce(
                            out=wm[:, nt, g * NWIN_G:(g + 1) * NWIN_G],
                            in_=ps[:, :GSIZES[g]].rearrange(
                                "p (w e) -> p w e", e=WIN),
                            axis=mybir.AxisListType.X,
                            op=mybir.AluOpType.max,
                        )
                    # top-8 windows of the whole 9216-wide m-row: scans just
                    # the 1152 window maxes instead of all R values again
                    nc.vector.max(out=vt[:, nt, :], in_=wm[:, nt, :])
                    nc.vector.max_index(
                        out=it[:, nt, :], in_max=vt[:, nt, :],
                        in_values=wm[:, nt, :])
                nc.gpsimd.dma_start(vals[b], vt)
                nc.gpsimd.dma_start(idxs[b], it)
    _dedupe_ldweights(nc)
    if fix_waits:
        _cap_sync_waits(nc)
    return nc


LAST_EXEC_NS = None
LAST_TRACE = None


def kernel(lrsr_lv2, ref_lv2):
    import os
    global LAST_EXEC_NS, LAST_TRACE
    from concourse.bass_utils import run_bass_kernel_spmd

    lr_u = _unfold(np.asarray(lrsr_lv2, dtype=np.float32))
    ref_u = _unfold(np.asarray(ref_lv2, dtype=np.float32))

    ref64 = ref_u.astype(np.float64)
    lr64 = lr_u.astype(np.float64)
    ref_nrm = np.maximum(np.sqrt((ref64 * ref64).sum(1)), EPS)  # [B, HW]
    lr_nrm = np.maximum(np.sqrt((lr64 * lr64).sum(1)), EPS)     # [B, HW]
    refn64 = ref64 / ref_nrm[:, None, :]

    refk = _to_chunks_f8(ref_u / ref_nrm[:, None, :].astype(np.float32), 16.0)
    lrk = _to_chunks_f8(lr_u, 1.0)

    core_ids = list(range(NCORES))
    in_maps = [
        {"refk": refk,
         "lrk": np.ascontiguousarray(lrk[:, :, :, :, c * NPC:(c + 1) * NPC])}
        for c in core_ids
    ]
    nc = _build_bass()
    trace = os.environ.get("KERNEL_TRACE", "0") == "1"
    out = run_bass_kernel_spmd(nc, in_maps, core_ids, trace=trace)
    res = out.results
    LAST_EXEC_NS = out.exec_time_ns
    if out.instructions_and_trace is not None:
        LAST_TRACE = out.instructions_and_trace[1]

    # [B, HW, NTOP] top-8 window ids per pixel, in global n order
    # (core-major, then n-tile, then partition); device layout is
    # [B, partition, n-tile, slot]
    idxs = np.concatenate(
        [res[c]["idxs"].transpose(0, 2, 1, 3).reshape(B, NPC, NTOP)
         for c in core_ids], axis=1).astype(np.int64)
    # unmatched MaxIndex slots (shouldn't happen) -> clamp to a valid window
    idxs = np.minimum(idxs, NWIN - 1)

    # expand each winning window to its WIN member positions and rescore:
    # stage A in fp32 (block-gather + batched BLAS gemm), stage B in fp64
    # for the exact winner + S value
    refn32 = refn64.astype(np.float32)
    lr32 = lr_u
    cand = (idxs[:, :, :, None] * WIN
            + np.arange(WIN)[None, None, None, :]).reshape(B, HW, NTOP * WIN)

    S = np.empty((B, HW), np.float32)
    Hm = np.empty((B, HW), np.int32)
    CH = 2048
    for b in range(B):
        # [NWIN, K, WIN]: whole ref windows as contiguous blocks
        refw = np.ascontiguousarray(
            refn32[b].reshape(K, NWIN, WIN).transpose(1, 0, 2))
        win = idxs[b]                                       # [HW, NTOP]
        scA = np.empty((HW, NTOP, WIN), np.float32)
        for n0 in range(0, HW, CH):
            lrT = lr32[b][:, n0:n0 + CH].T[:, None, :]      # [CH, 1, K]
            for j in range(NTOP):
                blocks = refw[win[n0:n0 + CH, j]]           # [CH, K, WIN]
                scA[n0:n0 + CH, j] = np.matmul(lrT, blocks)[:, 0]
        scA = scA.reshape(HW, NTOP * WIN)
        topj = np.argpartition(-scA, NTOP - 1, axis=1)[:, :NTOP]
        c8 = np.take_along_axis(cand[b], topj, axis=1)      # [HW, 8]
        for n0 in range(0, HW, CH):
            cc = c8[n0:n0 + CH]
            g64 = refn64[b][:, cc]                          # [K, CH, 8]
            sc = np.einsum("kcr,kc->cr", g64, lr64[b][:, n0:n0 + CH])
            j = np.argmax(sc, axis=1)
            ar = np.arange(cc.shape[0])
            S[b, n0:n0 + CH] = (sc[ar, j] / lr_nrm[b, n0:n0 + CH]).astype(
                np.float32)
            Hm[b, n0:n0 + CH] = cc[ar, j].astype(np.int32)

    return (S.reshape(B, 1, H, W), Hm.reshape(B, 1, H, W))


# revision 46
# speedup vs baseline: 1.1864x; 1.1864x over previous
"""Patch-correlation argmax (retrieval KNN) on 8 Trainium2 NeuronCores.

Pipeline:
  host:   3x3 unfold of both images -> [B, 576, 9216] patch matrices,
          l2-normalize the ref patches, cast to fp8e4m3 in the DoubleRow
          interleave layout (K padded 576->768 = 3 chunks of 256).
  device: shard the lr-patch axis (n) across 8 cores (no collectives).
          Each core computes R tiles [128 n x 512 m] as 3 PSUM-accumulated
          fp8 DoubleRow matmuls, collapses each 1536-wide PSUM scan group
          to 32-element window maxes with one segmented tensor_reduce
          pass, and keeps the top-8 windows per m-row via MAX8 +
          FIND_INDEX8 over the 288 window maxes. R never touches HBM.
  host:   rescore all 8x32 member positions of the winning windows
          (fp32 batched gemm, then fp64 on the top 8) -> exact S (max
          corr) + Hmap (argmax index). Only the true argmax's WINDOW must
          reach the device top-8, so fp8 matmul noise never flips the
          final answer.
"""

import numpy as np

B, C, H, W = 2, 64, 96, 96
HW = H * W            # 9216 patches per image
K = C * 9             # 576 contraction dim
KC = 3                # K chunks of 256 (DoubleRow; 576 zero-padded to 768)
NCORES = 8
NPC = HW // NCORES    # 1152 lr patches per core
NT = NPC // 128       # 9 n-tiles of 128 per core
MTILE = 512           # matmul free dim / PSUM bank
MT = HW // MTILE      # 18 m-tiles
# m-scan groups: each 1536-element (3 PSUM banks) group is collapsed by a
# single segmented tensor_reduce(max) into 8-element window maxes. Uniform
# groups keep the PE-fill / DVE-scan cadence steady so PE never stalls past
# the ~3.4us HAM window (a throttled PE runs matmuls 2x slower).
GSIZES = (1536,) * 6
GBASES = tuple(1536 * i for i in range(6))
NG = len(GSIZES)
WIN = 32              # window size of the segmented max
NWIN_G = 1536 // WIN  # 48 window maxes per scan group
NWIN = HW // WIN      # 1152 window maxes per m-row
NTOP = 8              # top windows kept per pixel (MAX8 output width)
EPS = 1e-12


def _unfold(x):
    """[B,C,H,W] f32 -> [B, C*9, H*W] matching F.unfold(kernel=3, padding=1)."""
    xp = np.zeros((B, C, H + 2, W + 2), np.float32)
    xp[:, :, 1:-1, 1:-1] = x
    out = np.empty((B, C, 9, HW), np.float32)
    for di in range(3):
        for dj in range(3):
            out[:, :, di * 3 + dj] = xp[:, :, di:di + H, dj:dj + W].reshape(B, C, HW)
    return out.reshape(B, K, HW)


def _to_chunks_f8(u, scale):
    """[B, 576, HW'] f32 -> [B, 128, KC, 2, HW'] fp8e4m3, K padded to 768.

    DoubleRow layout: cell (p, kc, j) holds original k = kc*256 + j*128 + p.
    `scale` keeps small components out of the e4m3 subnormal range; it
    multiplies R uniformly, which the argmax pipeline never looks past.
    """
    import ml_dtypes
    n = u.shape[2]
    up = np.zeros((B, KC * 256, n), np.float32)
    up[:, :K] = u * scale
    return np.ascontiguousarray(
        up.reshape(B, KC, 2, 128, n).transpose(0, 3, 1, 2, 4)
    ).astype(ml_dtypes.float8_e4m3)


def _dedupe_ldweights(nc):
    """Drop an InstLdweights when the PE array already holds those weights.

    bass emits one LDWEIGHTS per matmul; with kc as the outer loop the same
    lhsT serves 4 consecutive matmuls, and LDW+MM serialize on the PE queue
    (~107ns each), so redundant loads cost real time. Weight state persists
    in the array across matmuls; any waits/updates on a dropped LDW migrate
    to the next PE instruction (later wait = same guarantee, later update =
    still-ordered signal).
    """
    import concourse.mybir as mybir

    for blk in nc.m.functions[0].blocks:
        out, last_sig, pending = [], None, []
        changed = False
        for ins in blk.instructions:
            nm = type(ins).__name__
            if nm == "InstLdweights":
                sig = str(ins.ins[0])
                if sig == last_sig:
                    if ins.sync_info is not None:
                        pending.append(ins.sync_info)
                    changed = True
                    continue
                last_sig = sig
            if pending and nm in ("InstMatmult", "InstLdweights"):
                si = ins.sync_info
                if si is None:
                    ins.sync_info = si = mybir.SyncInfo(on_wait=[], on_update=[])
                for p in pending:
                    si.on_wait.extend(p.on_wait)
                    si.on_update.extend(p.on_update)
                pending = []
            out.append(ins)
        assert not pending
        if changed:
            blk.instructions = out


def _cap_sync_waits(nc):
    """Walrus instruction structs hold a single sync-wait slot (matmul, DMA).

    Tile sometimes emits 2 waits on one instruction (input RAW + slot WAR).
    Split the excess into standalone EventSemaphore waits inserted directly
    before the instruction on the same engine queue — semantically identical
    (every wait still holds at the same queue position), always compilable.
    """
    import concourse.mybir as mybir

    n = 0
    for blk in nc.m.functions[0].blocks:
        ins_list = list(blk.instructions)
        out, changed = [], False
        for ins in ins_list:
            si = ins.sync_info
            if (si is not None and len(si.on_wait) > 1
                    and type(ins).__name__ != "InstEventSemaphore"):
                for w in si.on_wait[:-1]:
                    n += 1
                    out.append(mybir.InstEventSemaphore(
                        name=f"bridge_wait_{n}",
                        engine=ins.engine,
                        sync_info=mybir.SyncInfo(on_wait=[w], on_update=[]),
                    ))
                si.on_wait[:] = si.on_wait[-1:]
                changed = True
            out.append(ins)
        if changed:
            blk.instructions = out


def _build_bass(fix_waits=True):
    import concourse.bass as bass
    import concourse.mybir as mybir
    from concourse.tile import TileContext

    nc = bass.Bass()
    refk = nc.declare_dram_parameter(
        "refk", [B, 128, KC, 2, HW], mybir.dt.float8e4, isOutput=False)
    lrk = nc.declare_dram_parameter(
        "lrk", [B, 128, KC, 2, NPC], mybir.dt.float8e4, isOutput=False)
    vals = nc.declare_dram_parameter(
        "vals", [B, 128, NT, NTOP], mybir.dt.float32, isOutput=True)
    idxs = nc.declare_dram_parameter(
        "idxs", [B, 128, NT, NTOP], mybir.dt.uint16, isOutput=True)

    with TileContext(nc) as tc:
        with (
            tc.tile_pool(name="big", bufs=1) as big,
            tc.tile_pool(name="io", bufs=2) as io,
            tc.tile_pool(name="lrp", bufs=2) as lrp,
            tc.tile_pool(name="wmp", bufs=1) as wmp,
            tc.tile_pool(name="ps", bufs=2, space="PSUM") as psp,
            tc.tile_pool(name="psw", bufs=1, space="PSUM") as psw,
        ):
            # ~6.5us of dummy matmuls on memset data while the first input
            # DMAs stream in: PE activity releases the HAM clock throttle
            # (cold PE runs matmuls at 1.2GHz instead of 2.4GHz), so the
            # real matmuls start warm instead of paying ~20us of ramp
            scratch = big.tile([128, MTILE], mybir.dt.float16, tag="warm")
            nc.gpsimd.memset(scratch[:], 0.0)
            wps = psw.tile([128, MTILE], mybir.dt.float32, tag="wps")
            for _ in range(20):
                nc.tensor.matmul(wps, scratch[:, :128], scratch[:],
                                 start=True, stop=True)
            for b in range(B):
                # Tile's dependency tracking is per-tile, so ref gets one
                # tile per m-slice (single DMA writer each) rather than 18
                # partial writes into one big tile: matmuls on slice mt can
                # start as soon as that slice has landed.
                # fp8 shrinks the inputs enough that BOTH batches' ref
                # and lr tiles fit in SBUF at once: distinct tiles per
                # batch, loaded once in need-order — no batch-boundary
                # reload, no WAR deps for the scheduler to trip over.
                lr_ts = [lrp.tile([128, 2, NPC], mybir.dt.float8e4,
                                  tag=f"lr{b}_{kc}", name=f"lr{b}_{kc}")
                         for kc in range(KC)]
                ref_ts = [big.tile([128, KC, 2, MTILE], mybir.dt.float8e4,
                                   tag=f"ref{b}_{mt}", name=f"ref{b}_{mt}")
                          for mt in range(MT)]
                for i in range(MT):
                    if i < KC:
                        nc.sync.dma_start(lr_ts[i][:], lrk[b, :, i, :, :])
                    sl = slice(i * MTILE, (i + 1) * MTILE)
                    nc.sync.dma_start(ref_ts[i][:], refk[b, :, :, :, sl])
                # one batch-wide candidate buffer + a single out-DMA per
                # tensor: the out-DMA then carries exactly one (RAW) wait,
                # and batch 1 gets a fresh slot (bufs=2) so there is no WAR
                # wait on top — DMA structs also allow only one sync wait
                vt = io.tile([128, NT, NTOP], mybir.dt.float32, tag="vals")
                it = io.tile([128, NT, NTOP], mybir.dt.uint16, tag="idx")
                # window maxes live on SBUF; written by tensor_reduce and
                # read by MAX8/FIND_INDEX8, all on the DVE queue, so the
                # whole reduction chain needs no cross-engine semaphores
                wm = wmp.tile([128, NT, NWIN], mybir.dt.float32, tag="wm")
                for nt in range(NT):
                    for g in range(NG):
                        # 3-bank PSUM scan group: matmuls fill the group's
                        # 512-wide bank slices, then one wide MAX8 +
                        # FIND_INDEX8 pair scans all of it. kc is the outer
                        # loop so one LDWEIGHTS serves all 3 bank slices.
                        ps = psp.tile([128, 1536], mybir.dt.float32, tag="ps")
                        nslices = GSIZES[g] // MTILE
                        for kc in range(KC):
                            for s in range(nslices):
                                mt = GBASES[g] // MTILE + s
                                nc.tensor.matmul(
                                    ps[:, s * MTILE:(s + 1) * MTILE],
                                    lr_ts[kc][:, :, nt * 128:(nt + 1) * 128],
                                    ref_ts[mt][:, kc, :, :],
                                    start=(kc == 0),
                                    stop=(kc == KC - 1),
                                    perf_mode=mybir.MatmulPerfMode.DoubleRow,
                                )
                        # one pass over the PSUM group -> 8-wide window
                        # maxes (segmented reduce over the innermost axis)
                        nc.vector.tensor_reduce(
                            out=wm[:, nt, g * NWIN_G:(g + 1) * NWIN_G],
                            in_=ps[:, :GSIZES[g]].rearrange(
                                "p (w e) -> p w e", e=WIN),
                            axis=mybir.AxisListType.X,
                            op=mybir.AluOpType.max,
                        )
                    # top-8 windows of the whole 9216-wide m-row: scans just
                    # the 1152 window maxes instead of all R values again
                    nc.vector.max(out=vt[:, nt, :], in_=wm[:, nt, :])
                    nc.vector.max_index(
                        out=it[:, nt, :], in_max=vt[:, nt, :],
                        in_values=wm[:, nt, :])
                nc.gpsimd.dma_start(vals[b], vt)
                nc.gpsimd.dma_start(idxs[b], it)
    _dedupe_ldweights(nc)
    if fix_waits:
        _cap_sync_waits(nc)
    return nc


LAST_EXEC_NS = None
LAST_TRACE = None


def kernel(lrsr_lv2, ref_lv2):
    import os
    global LAST_EXEC_NS, LAST_TRACE
    from concourse.bass_utils import run_bass_kernel_spmd

    lr_u = _unfold(np.asarray(lrsr_lv2, dtype=np.float32))
    ref_u = _unfold(np.asarray(ref_lv2, dtype=np.float32))

    ref64 = ref_u.astype(np.float64)
    lr64 = lr_u.astype(np.float64)
    ref_nrm = np.maximum(np.sqrt((ref64 * ref64).sum(1)), EPS)  # [B, HW]
    lr_nrm = np.maximum(np.sqrt((lr64 * lr64).sum(1)), EPS)     # [B, HW]
    refn64 = ref64 / ref_nrm[:, None, :]

    refk = _to_chunks_f8(ref_u / ref_nrm[:, None, :].astype(np.float32), 16.0)
    lrk = _to_chunks_f8(lr_u, 1.0)

    core_ids = list(range(NCORES))
    in_maps = [
        {"refk": refk,
         "lrk": np.ascontiguousarray(lrk[:, :, :, :, c * NPC:(c + 1) * NPC])}
        for c in core_ids
    ]
    nc = _build_bass()
    trace = os.environ.get("KERNEL_TRACE", "0") == "1"
    out = run_bass_kernel_spmd(nc, in_maps, core_ids, trace=trace)
    res = out.results
    LAST_EXEC_NS = out.exec_time_ns
    if out.instructions_and_trace is not None:
        LAST_TRACE = out.instructions_and_trace[1]

    # [B, HW, NTOP] top-8 window ids per pixel, in global n order
    # (core-major, then n-tile, then partition); device layout is
    # [B, partition, n-tile, slot]
    idxs = np.concatenate(
        [res[c]["idxs"].transpose(0, 2, 1, 3).reshape(B, NPC, NTOP)
         for c in core_ids], axis=1).astype(np.int64)
    # unmatched MaxIndex slots (shouldn't happen) -> clamp to a valid window
    idxs = np.minimum(idxs, NWIN - 1)

    # expand each winning window to its WIN member positions and rescore:
    # stage A in fp32 (block-gather + batched BLAS gemm), stage B in fp64
    # for the exact winner + S value
    refn32 = refn64.astype(np.float32)
    lr32 = lr_u
    cand = (idxs[:, :, :, None] * WIN
            + np.arange(WIN)[None, None, None, :]).reshape(B, HW, NTOP * WIN)

    S = np.empty((B, HW), np.float32)
    Hm = np.empty((B, HW), np.int32)
    CH = 2048
    for b in range(B):
        # [NWIN, K, WIN]: whole ref windows as contiguous blocks
        refw = np.ascontiguousarray(
            refn32[b].reshape(K, NWIN, WIN).transpose(1, 0, 2))
        win = idxs[b]                                       # [HW, NTOP]
        scA = np.empty((HW, NTOP, WIN), np.float32)
        for n0 in range(0, HW, CH):
            lrT = lr32[b][:, n0:n0 + CH].T[:, None, :]      # [CH, 1, K]
            for j in range(NTOP):
                blocks = refw[win[n0:n0 + CH, j]]           # [CH, K, WIN]
                scA[n0:n0 + CH, j] = np.matmul(lrT, blocks)[:, 0]
        scA = scA.reshape(HW, NTOP * WIN)
        topj = np.argpartition(-scA, NTOP - 1, axis=1)[:, :NTOP]
        c8 = np.take_along_axis(cand[b], topj, axis=1)      # [HW, 8]
        for n0 in range(0, HW, CH):
            cc = c8[n0:n0 + CH]
            g64 = refn64[b][:, cc]                          # [K, CH, 8]
            sc = np.einsum("kcr,kc->cr", g64, lr64[b][:, n0:n0 + CH])
            j = np.argmax(sc, axis=1)
            ar = np.arange(cc.shape[0])
            S[b, n0:n0 + CH] = (sc[ar, j] / lr_nrm[b, n0:n0 + CH]).astype(
                np.float32)
            Hm[b, n0:n0 + CH] = cc[ar, j].astype(np.int32)

    return (S.reshape(B, 1, H, W), Hm.reshape(B, 1, H, W))
